# revision 17
# baseline (speedup 1.0000x reference)
"""Causal GQA self-attention (B=4,T=2048,D=1024,H=16,HKV=4) on 8 trn2 cores.

Sharding: core c -> (batch b=c//2, head-half hh=c%2). Each core computes
8 query heads / 2 KV heads for one batch, plus the output projection
restricted to its 512 y-channels (full e). Host sums the two partial
projections per batch.

v2 redesign vs baseline:
 - x^T / Wq^T / Wkv^T / Wp^T are prepared host-side in bf16 (a sharding/
   layout choice), removing the on-device weight-transpose phase, all x
   transposes and their PSUM->SBUF copies, and halving input DMA bytes.
 - RMS-norm rsqrt is computed as exp(-0.5*ln(ms+eps)); ln and exp live in
   the same ACT table set (natural_log_exp_and_others), and one explicit
   LoadActFuncSet for that set is emitted up front, so the ACT stream has
   exactly one table load (the baseline's interleaved Sqrt/Exp caused 24
   loads = 31us). Norm stats are batched per 4-tile group: one Ln + one
   Exp instruction per group.
 - PSUM is repacked to exactly 8 banks: scores double-buffered (2x2) to
   keep the exp pipeline fed, single-buffered everything else (QKV,
   transposes, AV-y, proj), relying on interleaved emission so the PE
   always has other ready work.
 - AV accumulates both heads of a pair into one PSUM bank ([P,2,65]) and
   normalizes both with one reciprocal + one multiply.
 - Projection output staged PSUM->SBUF bf16 per 512-half, one DMA per
   half, interleaved into the next strip; bf16 partial outputs are summed
   on host in fp32.
 - Startup de-crunch: rope's cos/sin rotation split from the inv-rms
   multiply so it overlaps the stats; square ops on gpsimd; the first 8
   tiles' PSUM->SBUF copies on the (then idle) ACT engine; the last
   strip's projections ride per-tile behind its final-pair AV with PSUM
   alternating between two rings.
"""

import numpy as np

B, T, D = 4, 2048, 1024
H, HKV, HD = 16, 4, 64
P = 128
NT = T // P          # 16 t-tiles
DC = D // P          # 8 contraction chunks
HL = H // 2          # 8 local q heads
PAIRS = HL // 2      # 4 head pairs
NG = 4               # groups of 4 t-tiles (= query strips of 512)
ROPE_BASE = 10000.0
EPS = 1.1920928955078125e-07
SCALE = 1.0 / 8.0    # 1/sqrt(HD)
ACT_SET_LN_EXP = 6   # natural_log_exp_and_others in act_info.json

_CACHE = {}


def _rope_tables():
    inv = (1.0 / (ROPE_BASE ** (np.arange(0, HD, 2, dtype=np.float32) / HD))).astype(
        np.float32
    )
    t = np.arange(T, dtype=np.float32)
    f = np.outer(t, inv).astype(np.float32)
    return np.cos(f).astype(np.float32), np.sin(f).astype(np.float32)


def _build_program():
    import concourse.mybir as mybir
    import concourse.tile as tile
    from concourse import bacc
    from concourse.masks import make_identity, make_upper_triangular

    fp32 = mybir.dt.float32
    fp16 = mybir.dt.float16
    bf16 = mybir.dt.bfloat16
    AX = mybir.AxisListType.X
    MUL = mybir.AluOpType.mult
    ADD = mybir.AluOpType.add
    SUB = mybir.AluOpType.subtract
    EXP = mybir.ActivationFunctionType.Exp
    LN = mybir.ActivationFunctionType.Ln

    nc = bacc.Bacc("TRN2", target_bir_lowering=False, debug=False)

    xT_d = nc.dram_tensor("xT", [D, T], bf16, kind="ExternalInput").ap()
    wqT_d = nc.dram_tensor("wqT", [D, HL * HD], bf16, kind="ExternalInput").ap()
    wkvT_d = nc.dram_tensor("wkvT", [D, 4 * HD], bf16, kind="ExternalInput").ap()
    wpT_d = nc.dram_tensor("wpT", [HL * HD, D], bf16, kind="ExternalInput").ap()
    cos_d = nc.dram_tensor("cos", [T, HD // 2], bf16, kind="ExternalInput").ap()
    sin_d = nc.dram_tensor("sin", [T, HD // 2], bf16, kind="ExternalInput").ap()
    gain_d = nc.dram_tensor("gain", [P, HL], bf16, kind="ExternalInput").ap()
    out_d = nc.dram_tensor("out", [T, D], bf16, kind="ExternalOutput").ap()

    xT3 = xT_d.rearrange("(c p) t -> p c t", p=P)      # [128, DC, T]
    wqT3 = wqT_d.rearrange("(c p) e -> p c e", p=P)    # [128, DC, 512]
    wkvT3 = wkvT_d.rearrange("(c p) e -> p c e", p=P)  # [128, DC, 256]
    wpT3 = wpT_d.rearrange("(c p) e -> p c e", p=P)    # [128, PAIRS, D]
    out3 = out_d.rearrange("(n p) d -> n p d", p=P)

    with tile.TileContext(nc) as tc:
        with (
            tc.tile_pool(name="persist", bufs=1) as persist,
            tc.tile_pool(name="stageq", bufs=6) as stq,
            tc.tile_pool(name="stager", bufs=5) as str_,
            tc.tile_pool(name="stats", bufs=2) as stst,
            tc.tile_pool(name="p_pool", bufs=2) as pp,
            tc.tile_pool(name="yT_pool", bufs=2) as ytp,
            tc.tile_pool(name="stage_o", bufs=3) as seo,
            tc.tile_pool(name="stage_op", bufs=4) as sop,
            tc.tile_pool(name="small", bufs=8) as sm,
            tc.tile_pool(name="ps_att", bufs=2, space="PSUM") as psat,
            tc.tile_pool(name="ps_y", bufs=1, space="PSUM") as psy,
            tc.tile_pool(name="ps_qkv", bufs=1, space="PSUM") as psqkv,
            tc.tile_pool(name="ps_tp", bufs=1, space="PSUM") as pstp,
            tc.tile_pool(name="ps_o", bufs=1, space="PSUM") as pso,
        ):
            # ---- one-time ACT table load: ln+exp share set 6 ----
            ld = mybir.InstLoadActFuncSet(
                name=nc.get_next_instruction_name(), ins=[], outs=[],
                act_func_set_id=ACT_SET_LN_EXP,
            )
            ld.engine = mybir.EngineType.Activation
            nc.scalar.add_instruction(ld)

            # ---- constants ----
            ident = persist.tile([P, P], bf16)
            make_identity(nc, ident)
            dmask = persist.tile([P, P], bf16)
            make_upper_triangular(nc, dmask, val=1.0, diag=True)
            cos_sb = persist.tile([P, NT, HD // 2], bf16)
            sin_sb = persist.tile([P, NT, HD // 2], bf16)
            gain_sb = persist.tile([P, HL], bf16)
            eps_sb = persist.tile([P, 1], fp32)
            nc.vector.memset(eps_sb, EPS)

            # ---- persistent weights / activations ----
            xT_sb = persist.tile([P, DC, T], bf16)       # 32 KB/part
            wqT_sb = persist.tile([P, DC, HL * HD], bf16)
            wkvT_sb = persist.tile([P, DC, 4 * HD], bf16)
            wpT_sb = persist.tile([P, PAIRS, D], bf16)
            qT = persist.tile([P, PAIRS, T], bf16)       # [2-head dims, pair, t]
            kT2 = persist.tile([P, 2, T], bf16)          # kv heads, replicated halves
            v_sb = persist.tile([P, NT, 2 * (HD + 1)], bf16)  # ones-augmented
            y_nat = persist.tile([P, NT, HL * HD], bf16)

            v4 = v_sb.rearrange("p n (h x) -> p n h x", h=2)
            nc.gpsimd.memset(v4[:, :, :, HD : HD + 1], 1.0)

            # ---- input DMAs (HWDGE via sync/SP engine; ordered for startup) ----
            nc.sync.dma_start(xT_sb[:, 0:2, 0:P], xT3[:, 0:2, 0:P])
            nc.sync.dma_start(wqT_sb[:, 0:2, :], wqT3[:, 0:2, :])
            nc.sync.dma_start(xT_sb[:, 2:4, 0:P], xT3[:, 2:4, 0:P])
            nc.sync.dma_start(wqT_sb[:, 2:4, :], wqT3[:, 2:4, :])
            nc.sync.dma_start(xT_sb[:, 4:8, 0:P], xT3[:, 4:8, 0:P])
            nc.sync.dma_start(wqT_sb[:, 4:8, :], wqT3[:, 4:8, :])
            nc.sync.dma_start(wkvT_sb, wkvT3)
            nc.sync.dma_start(xT_sb[:, :, P : 2 * P], xT3[:, :, P : 2 * P])
            nc.sync.dma_start(cos_sb, cos_d.rearrange("(n p) c -> p n c", p=P))
            nc.sync.dma_start(sin_sb, sin_d.rearrange("(n p) c -> p n c", p=P))
            nc.sync.dma_start(gain_sb, gain_d)
            nc.sync.dma_start(xT_sb[:, :, 2 * P : 4 * P], xT3[:, :, 2 * P : 4 * P])
            for g in range(1, NG):
                t0 = g * 512
                nc.sync.dma_start(
                    xT_sb[:, :, t0 : t0 + 512], xT3[:, :, t0 : t0 + 512]
                )
            nc.sync.dma_start(wpT_sb, wpT3)

            # ================= phase units =================

            def q_unit(nt):
                hg, j = nt // 2, nt % 2
                ss = _ss_tiles[hg]
                fast = nt < 8  # startup region: spread work off the DVE
                # q: [t,e] = sum_d xT[d,t]*wqT[d,e]
                q_ps = psqkv.tile([P, HL * HD], fp32, tag="qkv", name=f"qp{nt}")
                for dc in range(DC):
                    nc.tensor.matmul(
                        q_ps, xT_sb[:, dc, nt * P : (nt + 1) * P],
                        wqT_sb[:, dc, :],
                        start=(dc == 0), stop=(dc == DC - 1),
                    )
                q_sb = stq.tile([P, HL * HD], bf16, tag="q_sb")
                _q_stage[nt] = q_sb
                if fast:
                    nc.scalar.copy(q_sb, q_ps)
                else:
                    nc.vector.tensor_copy(q_sb, q_ps)
                # stats on the (otherwise idle) gpsimd engine
                sq = str_.tile([P, HL * HD], bf16, tag="sq")
                nc.gpsimd.tensor_tensor(sq, q_sb, q_sb, MUL)
                nc.vector.reduce_sum(
                    ss[:, j, 0:HL],
                    sq.rearrange("p (h x) -> p h x", h=HL), axis=AX,
                )

            def kv_unit(nt):
                hg, j = nt // 2, nt % 2
                ss = _ss_tiles[hg]
                fast = nt < 8
                kv_ps = psqkv.tile([P, 4 * HD], fp32, tag="qkv", name=f"kvp{nt}")
                for dc in range(DC):
                    nc.tensor.matmul(
                        kv_ps, xT_sb[:, dc, nt * P : (nt + 1) * P],
                        wkvT_sb[:, dc, :],
                        start=(dc == 0), stop=(dc == DC - 1),
                    )
                k_sb = stq.tile([P, 2 * HD], bf16, tag="k_sb")
                _k_stage[nt] = k_sb
                if fast:
                    nc.scalar.copy(k_sb, kv_ps[:, 0 : 2 * HD])
                    nc.scalar.copy(
                        v4[:, nt, :, 0:HD],
                        kv_ps[:, 2 * HD : 4 * HD].rearrange(
                            "p (h x) -> p h x", h=2
                        ),
                    )
                else:
                    nc.vector.tensor_copy(k_sb, kv_ps[:, 0 : 2 * HD])
                    nc.vector.tensor_copy(
                        v4[:, nt, :, 0:HD],
                        kv_ps[:, 2 * HD : 4 * HD].rearrange(
                            "p (h x) -> p h x", h=2
                        ),
                    )
                sqk = str_.tile([P, 2 * HD], bf16, tag="sqk")
                nc.gpsimd.tensor_tensor(sqk, k_sb, k_sb, MUL)
                nc.vector.reduce_sum(
                    ss[:, j, HL : HL + 2],
                    sqk.rearrange("p (h x) -> p h x", h=2), axis=AX,
                )

            def lnexp_unit(hg):
                # inv = (ms+eps)^-1/2 = exp(-0.5*ln(ss/HD+eps)); q cols get gain
                ss = _ss_tiles[hg]
                lnt = stst.tile([P, 2, HL + 2], fp32, tag="lnt")
                nc.scalar.activation(
                    lnt, ss, LN, bias=eps_sb[:, 0:1], scale=1.0 / HD
                )
                inv = stst.tile([P, 2, HL + 2], bf16, tag="inv")
                _inv_tiles[hg] = inv
                nc.scalar.activation(inv, lnt, EXP, scale=-0.5)
                nc.vector.tensor_tensor(
                    inv[:, :, 0:HL], inv[:, :, 0:HL],
                    gain_sb[:, None, :].to_broadcast([P, 2, HL]), MUL,
                )

            def rope_cs(src, nh, nt, out, col0, eng=None):
                # cos/sin rotation only — independent of the norm stats, so
                # it runs while Pool computes them
                s3 = src.rearrange("p (h x) -> p h x", h=nh)
                h2 = HD // 2
                x1 = s3[:, :, 0:h2]
                x2 = s3[:, :, h2:HD]
                cb = cos_sb[:, nt : nt + 1, :].to_broadcast([P, nh, h2])
                sbr = sin_sb[:, nt : nt + 1, :].to_broadcast([P, nh, h2])
                r3 = out[:, col0 : col0 + nh * HD].rearrange(
                    "p (h x) -> p h x", h=nh
                )
                tmp = str_.tile([P, nh * h2], bf16, tag=f"t{nh}")
                t3 = tmp.rearrange("p (h x) -> p h x", h=nh)
                e = eng if eng is not None else nc.vector
                e.tensor_tensor(r3[:, :, 0:h2], x1, cb, MUL)
                e.tensor_tensor(t3, x2, sbr, MUL)
                e.tensor_tensor(r3[:, :, 0:h2], r3[:, :, 0:h2], t3, ADD)
                e.tensor_tensor(r3[:, :, h2:HD], x2, cb, MUL)
                e.tensor_tensor(t3, x1, sbr, MUL)
                e.tensor_tensor(
                    r3[:, :, h2:HD], r3[:, :, h2:HD], t3, SUB
                )

            def rope_cs_unit(nt):
                r = stq.tile([P, (HL + 2) * HD], bf16, tag="r",
                             name=f"r{nt}")
                _r_stage[nt] = r
                rope_cs(_q_stage[nt], HL, nt, r, 0)
                rope_cs(_k_stage[nt], 2, nt, r, HL * HD)

            def rope_tp_unit(nt):
                hg, j = nt // 2, nt % 2
                inv = _inv_tiles[hg]
                r = _r_stage.pop(nt)
                e = nc.vector
                r3q = r[:, 0 : HL * HD].rearrange("p (h x) -> p h x", h=HL)
                e.tensor_tensor(
                    r3q, r3q,
                    inv[:, j, 0:HL, None].to_broadcast([P, HL, HD]), MUL,
                )
                r3k = r[:, HL * HD :].rearrange("p (h x) -> p h x", h=2)
                e.tensor_tensor(
                    r3k, r3k,
                    inv[:, j, HL : HL + 2, None].to_broadcast([P, 2, HD]),
                    MUL,
                )
                # transposes: q pairs into cols 0:512, k (replicated) 512:768
                ps = pstp.tile([P, 6 * P], bf16, tag="tp")
                for pr in range(PAIRS):
                    nc.tensor.transpose(
                        ps[:, pr * P : (pr + 1) * P],
                        r[:, pr * P : (pr + 1) * P],
                        ident,
                    )
                for kv in range(2):
                    for rep in range(2):
                        nc.tensor.transpose(
                            ps[rep * 64 : (rep + 1) * 64,
                               4 * P + kv * P : 4 * P + (kv + 1) * P],
                            r[:, (HL + kv) * HD : (HL + kv + 1) * HD],
                            ident,
                            tile_position=(0, rep * 64),
                        )
                ce = nc.vector
                if ce is nc.scalar:
                    ce.copy(
                        qT[:, :, nt * P : (nt + 1) * P],
                        ps[:, 0 : 4 * P].rearrange("p (a b) -> p a b", b=P),
                    )
                    ce.copy(
                        kT2[:, :, nt * P : (nt + 1) * P],
                        ps[:, 4 * P : 6 * P].rearrange("p (a b) -> p a b", b=P),
                    )
                else:
                    ce.tensor_copy(
                        qT[:, :, nt * P : (nt + 1) * P],
                        ps[:, 0 : 4 * P].rearrange("p (a b) -> p a b", b=P),
                    )
                    ce.tensor_copy(
                        kT2[:, :, nt * P : (nt + 1) * P],
                        ps[:, 4 * P : 6 * P].rearrange("p (a b) -> p a b", b=P),
                    )

            def group_units(g):
                for hg in (2 * g, 2 * g + 1):
                    a, b = 2 * hg, 2 * hg + 1
                    yield lambda nt=a: q_unit(nt)
                    yield lambda nt=a: kv_unit(nt)
                    yield lambda nt=a: rope_cs_unit(nt)
                    yield lambda nt=b: q_unit(nt)
                    yield lambda nt=b: kv_unit(nt)
                    yield lambda nt=b: rope_cs_unit(nt)
                    yield lambda hg=hg: lnexp_unit(hg)
                    yield lambda nt=a: rope_tp_unit(nt)
                    yield lambda nt=b: rope_tp_unit(nt)

            def scores_units(s, pr, p_tiles):
                # Score tiles ride in PAIRS through one 4-bank PSUM slot
                # ([P, 2(tile), 2(head), 512]); full tiles get ONE merged exp
                # per pair (halves the ACT per-instruction init overhead),
                # diagonal tiles keep per-half exps (c0 varies).
                tq0 = s * 512
                kv = pr // 2
                state = {}
                for tkb in range(4 * s + 4):
                    def unit(tkb=tkb):
                        m = tkb - 4 * s
                        c0 = max(m, 0) * P
                        if "sc" not in state:
                            state["sc"] = psat.tile(
                                [P, 2, 2, 512], fp32, tag="sc", bufs=1,
                                name=f"sc{s}_{pr}_{tkb}",
                            )
                            state["pt"] = pp.tile(
                                [P, 2, 2, 512], bf16, tag=f"p{tkb // 2}",
                                name=f"pt{s}_{pr}_{tkb}",
                            )
                            state["m0"] = m
                            half = 0
                        else:
                            half = 1
                        sc = state["sc"]
                        pt = state["pt"]
                        p_tiles[tkb] = (pt, half)
                        for h01 in range(2):
                            hp = h01 * 64
                            nc.tensor.matmul(
                                sc[:, half, h01, c0:512],
                                kT2[hp : hp + 64, kv,
                                    tkb * P : (tkb + 1) * P],
                                qT[hp : hp + 64, pr,
                                   tq0 + c0 : tq0 + 512],
                                start=True, stop=True,
                            )
                        if m < 0:
                            if half == 1:
                                nc.scalar.activation(
                                    pt, sc, EXP, scale=SCALE,
                                )
                                state.clear()
                        else:
                            nc.scalar.activation(
                                pt[:, half, :, c0:512],
                                sc[:, half, :, c0:512],
                                EXP, scale=SCALE,
                            )
                            dm = dmask[:, None, :].to_broadcast([P, 2, P])
                            nc.gpsimd.tensor_tensor(
                                pt[:, half, :, c0 : c0 + P],
                                pt[:, half, :, c0 : c0 + P], dm, MUL,
                            )
                            if half == 1:
                                state.clear()
                    yield unit

            def av_units(s, pr, p_tiles):
                kv = pr // 2
                for tqi in range(4 * s, 4 * s + 4):
                    def unit(tqi=tqi):
                        co = (tqi - 4 * s) * P
                        y_ps = psy.tile([P, 2, HD + 1], fp32, tag="y")
                        for h01 in range(2):
                            for tkb in range(tqi + 1):
                                pt, half = p_tiles[tkb]
                                nc.tensor.matmul(
                                    y_ps[:, h01, :],
                                    pt[:, half, h01, co : co + P],
                                    v_sb[:, tkb,
                                         kv * (HD + 1) : (kv + 1) * (HD + 1)],
                                    start=(tkb == 0), stop=(tkb == tqi),
                                )
                        rcp = sm.tile([P, 2, 1], fp32, tag="rcp")
                        nc.vector.reciprocal(rcp, y_ps[:, :, HD : HD + 1])
                        nc.vector.tensor_tensor(
                            y_nat[:, tqi, 2 * pr * HD : (2 * pr + 2) * HD]
                                .rearrange("p (h x) -> p h x", h=2),
                            y_ps[:, :, 0:HD],
                            rcp.to_broadcast([P, 2, HD]),
                            MUL,
                        )
                    yield unit

            def proj_units(s, alt_pool=False, act_copy=False):
                # alt_pool: alternate proj PSUM between the pso ring and the
                # (idle by then) qkv ring so tail matmul groups double-buffer
                for j in range(4):
                    nt = 4 * s + j

                    def unit_t(nt=nt):
                        ps = pstp.tile([P, 6 * P], bf16, tag="tp")
                        for prr in range(PAIRS):
                            nc.tensor.transpose(
                                ps[:, prr * P : (prr + 1) * P],
                                y_nat[:, nt, prr * P : (prr + 1) * P],
                                ident,
                            )
                        yTs = ytp.tile([P, PAIRS, P], bf16, tag="yT")
                        _yT_stage[nt] = yTs
                        src = ps[:, 0 : 4 * P].rearrange("p (a b) -> p a b", b=P)
                        if act_copy:
                            nc.scalar.copy(yTs, src)
                        else:
                            nc.vector.tensor_copy(yTs, src)
                    yield unit_t

                    def unit_mm(nt=nt, j=j):
                        yTs = _yT_stage.pop(nt)
                        o_sb = seo.tile([P, D], bf16, tag="osb")
                        for ec in range(2):
                            if alt_pool and (2 * j + ec) % 2 == 1:
                                o_ps = psqkv.tile([P, 512], fp32, tag="qkv",
                                                  name=f"oq{nt}_{ec}")
                            else:
                                o_ps = pso.tile([P, 512], fp32, tag="o")
                            for prr in range(PAIRS):
                                nc.tensor.matmul(
                                    o_ps,
                                    yTs[:, prr, :],
                                    wpT_sb[:, prr, ec * 512 : (ec + 1) * 512],
                                    start=(prr == 0), stop=(prr == PAIRS - 1),
                                )
                            if act_copy:
                                nc.scalar.copy(
                                    o_sb[:, ec * 512 : (ec + 1) * 512], o_ps
                                )
                            else:
                                nc.vector.tensor_copy(
                                    o_sb[:, ec * 512 : (ec + 1) * 512], o_ps
                                )
                            nc.sync.dma_start(
                                out3[nt][:, ec * 512 : (ec + 1) * 512],
                                o_sb[:, ec * 512 : (ec + 1) * 512],
                            )
                    yield unit_mm

            def proj_part_units(s):
                # pairs 0-2 partial projection: runnable as soon as pair-2 AV
                # of the tile is done, i.e. during the ACT-saturated region,
                # so the tail only adds pair-3's rank-128 update
                for j in range(4):
                    nt = 4 * s + j

                    def unit_p(nt=nt, j=j):
                        ps = pstp.tile([P, 6 * P], bf16, tag="tp")
                        for prr in range(3):
                            nc.tensor.transpose(
                                ps[:, prr * P : (prr + 1) * P],
                                y_nat[:, nt, prr * P : (prr + 1) * P],
                                ident,
                            )
                        yT012 = ytp.tile([P, 3, P], bf16, tag="yT012")
                        nc.vector.tensor_copy(
                            yT012,
                            ps[:, 0 : 3 * P].rearrange("p (a b) -> p a b", b=P),
                        )
                        opart = sop.tile([P, D], bf16, tag="opart",
                                         name=f"op{nt}")
                        _opart_stage[nt] = opart
                        for ec in range(2):
                            if (2 * j + ec) % 2 == 1:
                                o_ps = psqkv.tile([P, 512], fp32, tag="qkv",
                                                  name=f"opq{nt}_{ec}")
                            else:
                                o_ps = pso.tile([P, 512], fp32, tag="o")
                            for prr in range(3):
                                nc.tensor.matmul(
                                    o_ps, yT012[:, prr, :],
                                    wpT_sb[:, prr, ec * 512 : (ec + 1) * 512],
                                    start=(prr == 0), stop=(prr == 2),
                                )
                            nc.vector.tensor_copy(
                                opart[:, ec * 512 : (ec + 1) * 512], o_ps
                            )
                    yield unit_p

            def proj_fin_units(s):
                for j in range(4):
                    nt = 4 * s + j

                    def unit_f(nt=nt, j=j):
                        ps = pstp.tile([P, 6 * P], bf16, tag="tp")
                        nc.tensor.transpose(
                            ps[:, 0:P], y_nat[:, nt, 3 * P : 4 * P], ident
                        )
                        yT3 = ytp.tile([P, P], bf16, tag="yT3")
                        nc.vector.tensor_copy(yT3, ps[:, 0:P])
                        opart = _opart_stage.pop(nt)
                        o_sb = seo.tile([P, D], bf16, tag="osb")
                        for ec in range(2):
                            if (2 * j + ec) % 2 == 1:
                                o_ps = psqkv.tile([P, 512], fp32, tag="qkv",
                                                  name=f"ofq{nt}_{ec}")
                            else:
                                o_ps = pso.tile([P, 512], fp32, tag="o")
                            nc.tensor.matmul(
                                o_ps, yT3,
                                wpT_sb[:, 3, ec * 512 : (ec + 1) * 512],
                                start=True, stop=True,
                            )
                            nc.vector.tensor_tensor(
                                o_sb[:, ec * 512 : (ec + 1) * 512],
                                o_ps, opart[:, ec * 512 : (ec + 1) * 512],
                                ADD,
                            )
                            nc.sync.dma_start(
                                out3[nt][:, ec * 512 : (ec + 1) * 512],
                                o_sb[:, ec * 512 : (ec + 1) * 512],
                            )
                    yield unit_f

            def merge_lead(primary, secondary):
                # proportionally interleave two lists, primary leading
                pu, su = list(primary), list(secondary)
                np_, ns_ = len(pu), len(su)
                out, si = [], 0
                for i, u in enumerate(pu):
                    out.append(u)
                    while si < ns_ and (si + 1) * np_ <= (i + 1) * ns_:
                        out.append(su[si])
                        si += 1
                out.extend(su[si:])
                return out

            def strip_primary(s, inline_proj):
                # scores(pair0), then per pair: scores(pr+1) zipped with
                # av(pr); for the last strip emitted, proj units ride right
                # behind the last pair's per-tile AV.
                lead2 = s == _first_strip  # two-pair lookahead in the exp-poor first strip
                units = []
                tiles = {0: {}}
                units += list(scores_units(s, 0, tiles[0]))
                if lead2:
                    # run two pairs of scores ahead of the AV wave in the
                    # exp-poor early strips
                    tiles[1] = {}
                    units += list(scores_units(s, 1, tiles[1]))
                for pr in range(PAIRS):
                    av = list(av_units(s, pr, tiles[pr]))
                    nx = pr + (2 if lead2 else 1)
                    if nx < PAIRS:
                        tiles[nx] = {}
                        sc = list(scores_units(s, nx, tiles[nx]))
                        units += merge_lead(sc, av)
                    if nx >= PAIRS and pr + 1 < PAIRS:
                        units += av
                    elif pr + 1 < PAIRS:
                        pass
                    elif inline_proj:
                        # pairs 0-2 partial projections first (run during the
                        # ACT-bound stretch), then av(j) with the pair-3
                        # finish trailing by one tile
                        pp_ = list(proj_part_units(s))
                        pf = list(proj_fin_units(s))
                        units.extend(pp_)
                        for j in range(4):
                            units.append(av[j])
                            if j > 0:
                                units.append(pf[j - 1])
                        units.append(pf[3])
                    else:
                        units += av
                return units

            def emit_with_extras(primary, extra_specs):
                # extra_specs: (unit, frac) — run unit after frac*len(primary)
                n = len(primary)
                at = {}
                for u, f in extra_specs:
                    at.setdefault(min(n, max(0, int(f * n))), []).append(u)
                for i, u in enumerate(primary):
                    for eu in at.get(i, ()):
                        eu()
                    u()
                for eu in at.get(n, ()):
                    eu()

            # ================= emission =================
            _q_stage, _k_stage, _yT_stage, _r_stage = {}, {}, {}, {}
            _opart_stage = {}
            _ss_tiles, _inv_tiles = {}, {}
            for hg in range(2 * NG):
                ss = stst.tile([P, 2, HL + 2], fp32, tag="ss", name=f"ss{hg}")
                _ss_tiles[hg] = ss

            # Strip order: big-exp strips in the middle so ACT saturates,
            # tiny strip 0 last for a short drain tail.
            order = [0, 1, 2, 3]
            _first_strip = order[0]
            for u in group_units(0):
                u()
            for u in group_units(1):
                u()
            prepped = 2
            prev = None
            for idx, s in enumerate(order):
                extras = []
                nxt = order[idx + 1] if idx + 1 < len(order) else None
                if nxt is not None and nxt + 1 > prepped:
                    gu = []
                    for g in range(prepped, nxt + 1):
                        gu.extend(group_units(g))
                    prepped = nxt + 1
                    for i, u in enumerate(gu):
                        extras.append((u, 0.15 + 0.6 * i / len(gu)))
                if prev is not None:
                    pu = list(proj_units(prev))
                    for i, u in enumerate(pu):
                        extras.append((u, 0.1 + 0.85 * i / len(pu)))
                emit_with_extras(
                    strip_primary(s, inline_proj=(idx == len(order) - 1)),
                    extras,
                )
                prev = s

    nc.compile()
    return nc


def _get_program():
    if "nc" not in _CACHE:
        _CACHE["nc"] = _build_program()
    return _CACHE["nc"]


def make_in_maps(x, Wq, Wk, Wv, Wproj, q_gain):
    import ml_dtypes

    bf16 = ml_dtypes.bfloat16
    cos, sin = _rope_tables()
    cos = cos.astype(bf16)
    sin = sin.astype(bf16)
    in_maps = []
    for c in range(8):
        b, hh = c // 2, c % 2
        wq = Wq[hh * 512 : (hh + 1) * 512]          # [512, 1024]
        wk = Wk[hh * 128 : (hh + 1) * 128]          # [128, 1024]
        wv = Wv[hh * 128 : (hh + 1) * 128]          # [128, 1024]
        wkv = np.concatenate([wk, wv], axis=0)      # [256, 1024]
        wp = Wproj[:, hh * 512 : (hh + 1) * 512]    # [1024, 512]
        in_maps.append(
            {
                "xT": np.ascontiguousarray(x[b].T).astype(bf16),
                "wqT": np.ascontiguousarray(wq.T).astype(bf16),
                "wkvT": np.ascontiguousarray(wkv.T).astype(bf16),
                "wpT": np.ascontiguousarray(wp.T).astype(bf16),
                "cos": cos,
                "sin": sin,
                "gain": np.ascontiguousarray(
                    np.broadcast_to(q_gain[hh * 8 : (hh + 1) * 8], (P, HL))
                ).astype(bf16),
            }
        )
    return in_maps


def kernel(x, Wq, Wk, Wv, Wproj, q_gain):
    from concourse import bass_utils

    x = np.asarray(x, dtype=np.float32)
    Wq = np.asarray(Wq, dtype=np.float32)
    Wk = np.asarray(Wk, dtype=np.float32)
    Wv = np.asarray(Wv, dtype=np.float32)
    Wproj = np.asarray(Wproj, dtype=np.float32)
    q_gain = np.asarray(q_gain, dtype=np.float32)

    nc = _get_program()
    in_maps = make_in_maps(x, Wq, Wk, Wv, Wproj, q_gain)
    res = bass_utils.run_bass_kernel_spmd(
        nc, in_maps, core_ids=list(range(8)), trace=False
    )
    out = np.empty((B, T, D), dtype=np.float32)
    for b in range(B):
        out[b] = np.asarray(res.results[2 * b]["out"], np.float32) + np.asarray(
            res.results[2 * b + 1]["out"], np.float32
        )
    return out



# revision 18
# speedup vs baseline: 1.3070x; 1.3070x over previous
"""Causal GQA self-attention (B=4,T=2048,D=1024,H=16,HKV=4) on 8 trn2 cores.

Sharding: core c -> (batch b=c//2, head-half hh=c%2). Each core computes
8 query heads / 2 KV heads for one batch, plus the output projection
restricted to its 512 y-channels (full e). Host sums the two partial
projections per batch.

v2 redesign vs baseline:
 - x^T / Wq^T / Wkv^T / Wp^T are prepared host-side in bf16 (a sharding/
   layout choice), removing the on-device weight-transpose phase, all x
   transposes and their PSUM->SBUF copies, and halving input DMA bytes.
 - RMS-norm rsqrt is computed as exp(-0.5*ln(ms+eps)); ln and exp live in
   the same ACT table set (natural_log_exp_and_others), and one explicit
   LoadActFuncSet for that set is emitted up front, so the ACT stream has
   exactly one table load (the baseline's interleaved Sqrt/Exp caused 24
   loads = 31us). Norm stats are batched per 4-tile group: one Ln + one
   Exp instruction per group.
 - PSUM is repacked to exactly 8 banks: scores double-buffered (2x2) to
   keep the exp pipeline fed, single-buffered everything else (QKV,
   transposes, AV-y, proj), relying on interleaved emission so the PE
   always has other ready work.
 - AV accumulates both heads of a pair into one PSUM bank ([P,2,65]) and
   normalizes both with one reciprocal + one multiply.
 - Projection output staged PSUM->SBUF bf16 per 512-half, one DMA per
   half, interleaved into the next strip; bf16 partial outputs are summed
   on host in fp32.
 - Startup de-crunch: rope's cos/sin rotation split from the inv-rms
   multiply so it overlaps the stats; square ops on gpsimd; the first 8
   tiles' PSUM->SBUF copies on the (then idle) ACT engine; the last
   strip's projections ride per-tile behind its final-pair AV with PSUM
   alternating between two rings.
"""

import numpy as np

B, T, D = 4, 2048, 1024
H, HKV, HD = 16, 4, 64
P = 128
NT = T // P          # 16 t-tiles
DC = D // P          # 8 contraction chunks
HL = H // 2          # 8 local q heads
PAIRS = HL // 2      # 4 head pairs
NG = 4               # groups of 4 t-tiles (= query strips of 512)
ROPE_BASE = 10000.0
EPS = 1.1920928955078125e-07
SCALE = 1.0 / 8.0    # 1/sqrt(HD)
ACT_SET_LN_EXP = 6   # natural_log_exp_and_others in act_info.json

_CACHE = {}


def _rope_tables():
    inv = (1.0 / (ROPE_BASE ** (np.arange(0, HD, 2, dtype=np.float32) / HD))).astype(
        np.float32
    )
    t = np.arange(T, dtype=np.float32)
    f = np.outer(t, inv).astype(np.float32)
    return np.cos(f).astype(np.float32), np.sin(f).astype(np.float32)


def _build_program():
    import concourse.mybir as mybir
    import concourse.tile as tile
    from concourse import bacc
    from concourse.masks import make_identity, make_upper_triangular

    fp32 = mybir.dt.float32
    fp16 = mybir.dt.float16
    bf16 = mybir.dt.bfloat16
    AX = mybir.AxisListType.X
    MUL = mybir.AluOpType.mult
    ADD = mybir.AluOpType.add
    SUB = mybir.AluOpType.subtract
    EXP = mybir.ActivationFunctionType.Exp
    LN = mybir.ActivationFunctionType.Ln

    nc = bacc.Bacc("TRN2", target_bir_lowering=False, debug=False)

    xT_d = nc.dram_tensor("xT", [D, T], bf16, kind="ExternalInput").ap()
    wqT_d = nc.dram_tensor("wqT", [D, HL * HD], bf16, kind="ExternalInput").ap()
    wkvT_d = nc.dram_tensor("wkvT", [D, 4 * HD], bf16, kind="ExternalInput").ap()
    wpT_d = nc.dram_tensor("wpT", [HL * HD, D], bf16, kind="ExternalInput").ap()
    cos_d = nc.dram_tensor("cos", [T, HD // 2], bf16, kind="ExternalInput").ap()
    sin_d = nc.dram_tensor("sin", [T, HD // 2], bf16, kind="ExternalInput").ap()
    gain_d = nc.dram_tensor("gain", [P, HL], bf16, kind="ExternalInput").ap()
    out_d = nc.dram_tensor("out", [T, D], bf16, kind="ExternalOutput").ap()

    xT3 = xT_d.rearrange("(c p) t -> p c t", p=P)      # [128, DC, T]
    wqT3 = wqT_d.rearrange("(c p) e -> p c e", p=P)    # [128, DC, 512]
    wkvT3 = wkvT_d.rearrange("(c p) e -> p c e", p=P)  # [128, DC, 256]
    wpT3 = wpT_d.rearrange("(c p) e -> p c e", p=P)    # [128, PAIRS, D]
    out3 = out_d.rearrange("(n p) d -> n p d", p=P)

    with tile.TileContext(nc) as tc:
        with (
            tc.tile_pool(name="persist", bufs=1) as persist,
            tc.tile_pool(name="stageq", bufs=6) as stq,
            tc.tile_pool(name="stager", bufs=5) as str_,
            tc.tile_pool(name="stats", bufs=2) as stst,
            tc.tile_pool(name="p_pool", bufs=2) as pp,
            tc.tile_pool(name="yT_pool", bufs=2) as ytp,
            tc.tile_pool(name="stage_o", bufs=3) as seo,
            tc.tile_pool(name="stage_op", bufs=4) as sop,
            tc.tile_pool(name="small", bufs=8) as sm,
            tc.tile_pool(name="ps_att", bufs=2, space="PSUM") as psat,
            tc.tile_pool(name="ps_y", bufs=1, space="PSUM") as psy,
            tc.tile_pool(name="ps_qkv", bufs=1, space="PSUM") as psqkv,
            tc.tile_pool(name="ps_tp", bufs=1, space="PSUM") as pstp,
            tc.tile_pool(name="ps_o", bufs=1, space="PSUM") as pso,
        ):
            # ---- one-time ACT table load: ln+exp share set 6 ----
            ld = mybir.InstLoadActFuncSet(
                name=nc.get_next_instruction_name(), ins=[], outs=[],
                act_func_set_id=ACT_SET_LN_EXP,
            )
            ld.engine = mybir.EngineType.Activation
            nc.scalar.add_instruction(ld)

            # ---- constants ----
            ident = persist.tile([P, P], bf16)
            make_identity(nc, ident)
            dmask = persist.tile([P, P], bf16)
            make_upper_triangular(nc, dmask, val=1.0, diag=True)
            cos_sb = persist.tile([P, NT, HD // 2], bf16)
            sin_sb = persist.tile([P, NT, HD // 2], bf16)
            gain_sb = persist.tile([P, HL], bf16)
            eps_sb = persist.tile([P, 1], fp32)
            nc.vector.memset(eps_sb, EPS)

            # ---- persistent weights / activations ----
            xT_sb = persist.tile([P, DC, T], bf16)       # 32 KB/part
            wqT_sb = persist.tile([P, DC, HL * HD], bf16)
            wkvT_sb = persist.tile([P, DC, 4 * HD], bf16)
            wpT_sb = persist.tile([P, PAIRS, D], bf16)
            qT = persist.tile([P, PAIRS, T], bf16)       # [2-head dims, pair, t]
            kT2 = persist.tile([P, 2, T], bf16)          # kv heads, replicated halves
            v_sb = persist.tile([P, NT, 2 * (HD + 1)], bf16)  # ones-augmented
            y_nat = persist.tile([P, NT, HL * HD], bf16)

            v4 = v_sb.rearrange("p n (h x) -> p n h x", h=2)
            nc.gpsimd.memset(v4[:, :, :, HD : HD + 1], 1.0)

            # ---- input DMAs (HWDGE via sync/SP engine; ordered for startup) ----
            nc.sync.dma_start(xT_sb[:, 0:2, 0:P], xT3[:, 0:2, 0:P])
            nc.sync.dma_start(wqT_sb[:, 0:2, :], wqT3[:, 0:2, :])
            nc.sync.dma_start(xT_sb[:, 2:4, 0:P], xT3[:, 2:4, 0:P])
            nc.sync.dma_start(wqT_sb[:, 2:4, :], wqT3[:, 2:4, :])
            nc.sync.dma_start(xT_sb[:, 4:8, 0:P], xT3[:, 4:8, 0:P])
            nc.sync.dma_start(wqT_sb[:, 4:8, :], wqT3[:, 4:8, :])
            nc.sync.dma_start(wkvT_sb, wkvT3)
            nc.sync.dma_start(xT_sb[:, :, P : 2 * P], xT3[:, :, P : 2 * P])
            nc.sync.dma_start(cos_sb, cos_d.rearrange("(n p) c -> p n c", p=P))
            nc.sync.dma_start(sin_sb, sin_d.rearrange("(n p) c -> p n c", p=P))
            nc.sync.dma_start(gain_sb, gain_d)
            nc.sync.dma_start(xT_sb[:, :, 2 * P : 4 * P], xT3[:, :, 2 * P : 4 * P])
            for g in range(1, NG):
                t0 = g * 512
                nc.sync.dma_start(
                    xT_sb[:, :, t0 : t0 + 512], xT3[:, :, t0 : t0 + 512]
                )
            nc.sync.dma_start(wpT_sb, wpT3)

            # ================= phase units =================

            def q_unit(nt):
                hg, j = nt // 2, nt % 2
                ss = _ss_tiles[hg]
                fast = nt < 8  # startup region: spread work off the DVE
                # q: [t,e] = sum_d xT[d,t]*wqT[d,e]
                q_ps = psqkv.tile([P, HL * HD], fp32, tag="qkv", name=f"qp{nt}")
                for dc in range(DC):
                    nc.tensor.matmul(
                        q_ps, xT_sb[:, dc, nt * P : (nt + 1) * P],
                        wqT_sb[:, dc, :],
                        start=(dc == 0), stop=(dc == DC - 1),
                    )
                q_sb = stq.tile([P, HL * HD], bf16, tag="q_sb")
                _q_stage[nt] = q_sb
                if fast:
                    nc.scalar.copy(q_sb, q_ps)
                else:
                    nc.vector.tensor_copy(q_sb, q_ps)
                # stats on the (otherwise idle) gpsimd engine
                sq = str_.tile([P, HL * HD], bf16, tag="sq")
                nc.gpsimd.tensor_tensor(sq, q_sb, q_sb, MUL)
                nc.vector.reduce_sum(
                    ss[:, j, 0:HL],
                    sq.rearrange("p (h x) -> p h x", h=HL), axis=AX,
                )

            def kv_unit(nt):
                hg, j = nt // 2, nt % 2
                ss = _ss_tiles[hg]
                fast = nt < 8
                kv_ps = psqkv.tile([P, 4 * HD], fp32, tag="qkv", name=f"kvp{nt}")
                for dc in range(DC):
                    nc.tensor.matmul(
                        kv_ps, xT_sb[:, dc, nt * P : (nt + 1) * P],
                        wkvT_sb[:, dc, :],
                        start=(dc == 0), stop=(dc == DC - 1),
                    )
                k_sb = stq.tile([P, 2 * HD], bf16, tag="k_sb")
                _k_stage[nt] = k_sb
                if fast:
                    nc.scalar.copy(k_sb, kv_ps[:, 0 : 2 * HD])
                    nc.scalar.copy(
                        v4[:, nt, :, 0:HD],
                        kv_ps[:, 2 * HD : 4 * HD].rearrange(
                            "p (h x) -> p h x", h=2
                        ),
                    )
                else:
                    nc.vector.tensor_copy(k_sb, kv_ps[:, 0 : 2 * HD])
                    nc.vector.tensor_copy(
                        v4[:, nt, :, 0:HD],
                        kv_ps[:, 2 * HD : 4 * HD].rearrange(
                            "p (h x) -> p h x", h=2
                        ),
                    )
                sqk = str_.tile([P, 2 * HD], bf16, tag="sqk")
                nc.gpsimd.tensor_tensor(sqk, k_sb, k_sb, MUL)
                nc.vector.reduce_sum(
                    ss[:, j, HL : HL + 2],
                    sqk.rearrange("p (h x) -> p h x", h=2), axis=AX,
                )

            def lnexp_unit(hg):
                # inv = (ms+eps)^-1/2 = exp(-0.5*ln(ss/HD+eps)); q cols get gain
                ss = _ss_tiles[hg]
                lnt = stst.tile([P, 2, HL + 2], fp32, tag="lnt")
                nc.scalar.activation(
                    lnt, ss, LN, bias=eps_sb[:, 0:1], scale=1.0 / HD
                )
                inv = stst.tile([P, 2, HL + 2], bf16, tag="inv")
                _inv_tiles[hg] = inv
                nc.scalar.activation(inv, lnt, EXP, scale=-0.5)
                nc.vector.tensor_tensor(
                    inv[:, :, 0:HL], inv[:, :, 0:HL],
                    gain_sb[:, None, :].to_broadcast([P, 2, HL]), MUL,
                )

            def rope_cs(src, nh, nt, out, col0, eng=None):
                # cos/sin rotation only — independent of the norm stats, so
                # it runs while Pool computes them
                s3 = src.rearrange("p (h x) -> p h x", h=nh)
                h2 = HD // 2
                x1 = s3[:, :, 0:h2]
                x2 = s3[:, :, h2:HD]
                cb = cos_sb[:, nt : nt + 1, :].to_broadcast([P, nh, h2])
                sbr = sin_sb[:, nt : nt + 1, :].to_broadcast([P, nh, h2])
                r3 = out[:, col0 : col0 + nh * HD].rearrange(
                    "p (h x) -> p h x", h=nh
                )
                tmp = str_.tile([P, nh * h2], bf16, tag=f"t{nh}")
                t3 = tmp.rearrange("p (h x) -> p h x", h=nh)
                e = eng if eng is not None else nc.vector
                e.tensor_tensor(r3[:, :, 0:h2], x1, cb, MUL)
                e.tensor_tensor(t3, x2, sbr, MUL)
                e.tensor_tensor(r3[:, :, 0:h2], r3[:, :, 0:h2], t3, ADD)
                e.tensor_tensor(r3[:, :, h2:HD], x2, cb, MUL)
                e.tensor_tensor(t3, x1, sbr, MUL)
                e.tensor_tensor(
                    r3[:, :, h2:HD], r3[:, :, h2:HD], t3, SUB
                )

            def rope_cs_unit(nt):
                r = stq.tile([P, (HL + 2) * HD], bf16, tag="r",
                             name=f"r{nt}")
                _r_stage[nt] = r
                rope_cs(_q_stage[nt], HL, nt, r, 0)
                rope_cs(_k_stage[nt], 2, nt, r, HL * HD)

            def rope_tp_unit(nt):
                hg, j = nt // 2, nt % 2
                inv = _inv_tiles[hg]
                r = _r_stage.pop(nt)
                e = nc.vector
                r3q = r[:, 0 : HL * HD].rearrange("p (h x) -> p h x", h=HL)
                e.tensor_tensor(
                    r3q, r3q,
                    inv[:, j, 0:HL, None].to_broadcast([P, HL, HD]), MUL,
                )
                r3k = r[:, HL * HD :].rearrange("p (h x) -> p h x", h=2)
                e.tensor_tensor(
                    r3k, r3k,
                    inv[:, j, HL : HL + 2, None].to_broadcast([P, 2, HD]),
                    MUL,
                )
                # transposes: q pairs into cols 0:512, k (replicated) 512:768
                ps = pstp.tile([P, 6 * P], bf16, tag="tp")
                for pr in range(PAIRS):
                    nc.tensor.transpose(
                        ps[:, pr * P : (pr + 1) * P],
                        r[:, pr * P : (pr + 1) * P],
                        ident,
                    )
                for kv in range(2):
                    for rep in range(2):
                        nc.tensor.transpose(
                            ps[rep * 64 : (rep + 1) * 64,
                               4 * P + kv * P : 4 * P + (kv + 1) * P],
                            r[:, (HL + kv) * HD : (HL + kv + 1) * HD],
                            ident,
                            tile_position=(0, rep * 64),
                        )
                ce = nc.vector
                if ce is nc.scalar:
                    ce.copy(
                        qT[:, :, nt * P : (nt + 1) * P],
                        ps[:, 0 : 4 * P].rearrange("p (a b) -> p a b", b=P),
                    )
                    ce.copy(
                        kT2[:, :, nt * P : (nt + 1) * P],
                        ps[:, 4 * P : 6 * P].rearrange("p (a b) -> p a b", b=P),
                    )
                else:
                    ce.tensor_copy(
                        qT[:, :, nt * P : (nt + 1) * P],
                        ps[:, 0 : 4 * P].rearrange("p (a b) -> p a b", b=P),
                    )
                    ce.tensor_copy(
                        kT2[:, :, nt * P : (nt + 1) * P],
                        ps[:, 4 * P : 6 * P].rearrange("p (a b) -> p a b", b=P),
                    )

            def group_units(g):
                for hg in (2 * g, 2 * g + 1):
                    a, b = 2 * hg, 2 * hg + 1
                    yield lambda nt=a: q_unit(nt)
                    yield lambda nt=a: kv_unit(nt)
                    yield lambda nt=a: rope_cs_unit(nt)
                    yield lambda nt=b: q_unit(nt)
                    yield lambda nt=b: kv_unit(nt)
                    yield lambda nt=b: rope_cs_unit(nt)
                    yield lambda hg=hg: lnexp_unit(hg)
                    yield lambda nt=a: rope_tp_unit(nt)
                    yield lambda nt=b: rope_tp_unit(nt)

            def scores_units(s, pr, p_tiles):
                tq0 = s * 512
                kv = pr // 2
                for tkb in range(4 * s + 4):
                    def unit(tkb=tkb):
                        m = tkb - 4 * s
                        c0 = max(m, 0) * P
                        pt = pp.tile([P, 2, 512], bf16, tag=f"p{tkb}")
                        p_tiles[tkb] = pt
                        sc = psat.tile([P, 2, 512], fp32, tag="sc")
                        for h01 in range(2):
                            hp = h01 * 64
                            nc.tensor.matmul(
                                sc[:, h01, c0:512],
                                kT2[hp : hp + 64, kv,
                                    tkb * P : (tkb + 1) * P],
                                qT[hp : hp + 64, pr,
                                   tq0 + c0 : tq0 + 512],
                                start=True, stop=True,
                            )
                        nc.scalar.activation(
                            pt[:, :, c0:512], sc[:, :, c0:512],
                            EXP, scale=SCALE,
                        )
                        if m >= 0:
                            dm = dmask[:, None, :].to_broadcast([P, 2, P])
                            nc.gpsimd.tensor_tensor(
                                pt[:, :, c0 : c0 + P],
                                pt[:, :, c0 : c0 + P], dm, MUL,
                            )
                    yield unit

            def av_units(s, pr, p_tiles):
                kv = pr // 2
                for tqi in range(4 * s, 4 * s + 4):
                    def unit(tqi=tqi):
                        co = (tqi - 4 * s) * P
                        y_ps = psy.tile([P, 2, HD + 1], fp32, tag="y")
                        for h01 in range(2):
                            for tkb in range(tqi + 1):
                                nc.tensor.matmul(
                                    y_ps[:, h01, :],
                                    p_tiles[tkb][:, h01, co : co + P],
                                    v_sb[:, tkb,
                                         kv * (HD + 1) : (kv + 1) * (HD + 1)],
                                    start=(tkb == 0), stop=(tkb == tqi),
                                )
                        rcp = sm.tile([P, 2, 1], fp32, tag="rcp")
                        nc.vector.reciprocal(rcp, y_ps[:, :, HD : HD + 1])
                        nc.vector.tensor_tensor(
                            y_nat[:, tqi, 2 * pr * HD : (2 * pr + 2) * HD]
                                .rearrange("p (h x) -> p h x", h=2),
                            y_ps[:, :, 0:HD],
                            rcp.to_broadcast([P, 2, HD]),
                            MUL,
                        )
                    yield unit

            def proj_units(s, alt_pool=False, act_copy=False):
                # alt_pool: alternate proj PSUM between the pso ring and the
                # (idle by then) qkv ring so tail matmul groups double-buffer
                for j in range(4):
                    nt = 4 * s + j

                    def unit_t(nt=nt):
                        ps = pstp.tile([P, 6 * P], bf16, tag="tp")
                        for prr in range(PAIRS):
                            nc.tensor.transpose(
                                ps[:, prr * P : (prr + 1) * P],
                                y_nat[:, nt, prr * P : (prr + 1) * P],
                                ident,
                            )
                        yTs = ytp.tile([P, PAIRS, P], bf16, tag="yT")
                        _yT_stage[nt] = yTs
                        src = ps[:, 0 : 4 * P].rearrange("p (a b) -> p a b", b=P)
                        if act_copy:
                            nc.scalar.copy(yTs, src)
                        else:
                            nc.vector.tensor_copy(yTs, src)
                    yield unit_t

                    def unit_mm(nt=nt, j=j):
                        yTs = _yT_stage.pop(nt)
                        o_sb = seo.tile([P, D], bf16, tag="osb")
                        for ec in range(2):
                            if alt_pool and (2 * j + ec) % 2 == 1:
                                o_ps = psqkv.tile([P, 512], fp32, tag="qkv",
                                                  name=f"oq{nt}_{ec}")
                            else:
                                o_ps = pso.tile([P, 512], fp32, tag="o")
                            for prr in range(PAIRS):
                                nc.tensor.matmul(
                                    o_ps,
                                    yTs[:, prr, :],
                                    wpT_sb[:, prr, ec * 512 : (ec + 1) * 512],
                                    start=(prr == 0), stop=(prr == PAIRS - 1),
                                )
                            if act_copy:
                                nc.scalar.copy(
                                    o_sb[:, ec * 512 : (ec + 1) * 512], o_ps
                                )
                            else:
                                nc.vector.tensor_copy(
                                    o_sb[:, ec * 512 : (ec + 1) * 512], o_ps
                                )
                            nc.sync.dma_start(
                                out3[nt][:, ec * 512 : (ec + 1) * 512],
                                o_sb[:, ec * 512 : (ec + 1) * 512],
                            )
                    yield unit_mm

            def proj_part_units(s):
                # pairs 0-2 partial projection: runnable as soon as pair-2 AV
                # of the tile is done, i.e. during the ACT-saturated region,
                # so the tail only adds pair-3's rank-128 update
                for j in range(4):
                    nt = 4 * s + j

                    def unit_p(nt=nt, j=j):
                        ps = pstp.tile([P, 6 * P], bf16, tag="tp")
                        for prr in range(3):
                            nc.tensor.transpose(
                                ps[:, prr * P : (prr + 1) * P],
                                y_nat[:, nt, prr * P : (prr + 1) * P],
                                ident,
                            )
                        yT012 = ytp.tile([P, 3, P], bf16, tag="yT012")
                        nc.vector.tensor_copy(
                            yT012,
                            ps[:, 0 : 3 * P].rearrange("p (a b) -> p a b", b=P),
                        )
                        opart = sop.tile([P, D], bf16, tag="opart",
                                         name=f"op{nt}")
                        _opart_stage[nt] = opart
                        for ec in range(2):
                            if (2 * j + ec) % 2 == 1:
                                o_ps = psqkv.tile([P, 512], fp32, tag="qkv",
                                                  name=f"opq{nt}_{ec}")
                            else:
                                o_ps = pso.tile([P, 512], fp32, tag="o")
                            for prr in range(3):
                                nc.tensor.matmul(
                                    o_ps, yT012[:, prr, :],
                                    wpT_sb[:, prr, ec * 512 : (ec + 1) * 512],
                                    start=(prr == 0), stop=(prr == 2),
                                )
                            nc.vector.tensor_copy(
                                opart[:, ec * 512 : (ec + 1) * 512], o_ps
                            )
                    yield unit_p

            def proj_fin_units(s):
                for j in range(4):
                    nt = 4 * s + j

                    def unit_f(nt=nt, j=j):
                        ps = pstp.tile([P, 6 * P], bf16, tag="tp")
                        nc.tensor.transpose(
                            ps[:, 0:P], y_nat[:, nt, 3 * P : 4 * P], ident
                        )
                        yT3 = ytp.tile([P, P], bf16, tag="yT3")
                        nc.vector.tensor_copy(yT3, ps[:, 0:P])
                        opart = _opart_stage.pop(nt)
                        o_sb = seo.tile([P, D], bf16, tag="osb")
                        for ec in range(2):
                            if (2 * j + ec) % 2 == 1:
                                o_ps = psqkv.tile([P, 512], fp32, tag="qkv",
                                                  name=f"ofq{nt}_{ec}")
                            else:
                                o_ps = pso.tile([P, 512], fp32, tag="o")
                            nc.tensor.matmul(
                                o_ps, yT3,
                                wpT_sb[:, 3, ec * 512 : (ec + 1) * 512],
                                start=True, stop=True,
                            )
                            nc.vector.tensor_tensor(
                                o_sb[:, ec * 512 : (ec + 1) * 512],
                                o_ps, opart[:, ec * 512 : (ec + 1) * 512],
                                ADD,
                            )
                            nc.sync.dma_start(
                                out3[nt][:, ec * 512 : (ec + 1) * 512],
                                o_sb[:, ec * 512 : (ec + 1) * 512],
                            )
                    yield unit_f

            def merge_lead(primary, secondary):
                # proportionally interleave two lists, primary leading
                pu, su = list(primary), list(secondary)
                np_, ns_ = len(pu), len(su)
                out, si = [], 0
                for i, u in enumerate(pu):
                    out.append(u)
                    while si < ns_ and (si + 1) * np_ <= (i + 1) * ns_:
                        out.append(su[si])
                        si += 1
                out.extend(su[si:])
                return out

            def strip_primary(s, inline_proj):
                # scores(pair0), then per pair: scores(pr+1) zipped with
                # av(pr); for the last strip emitted, proj units ride right
                # behind the last pair's per-tile AV.
                lead2 = s == _first_strip  # two-pair lookahead in the exp-poor first strip
                units = []
                tiles = {0: {}}
                units += list(scores_units(s, 0, tiles[0]))
                if lead2:
                    # run two pairs of scores ahead of the AV wave in the
                    # exp-poor early strips
                    tiles[1] = {}
                    units += list(scores_units(s, 1, tiles[1]))
                for pr in range(PAIRS):
                    av = list(av_units(s, pr, tiles[pr]))
                    nx = pr + (2 if lead2 else 1)
                    if nx < PAIRS:
                        tiles[nx] = {}
                        sc = list(scores_units(s, nx, tiles[nx]))
                        units += merge_lead(sc, av)
                    if nx >= PAIRS and pr + 1 < PAIRS:
                        units += av
                    elif pr + 1 < PAIRS:
                        pass
                    elif inline_proj:
                        # pairs 0-2 partial projections first (run during the
                        # ACT-bound stretch), then av(j) with the pair-3
                        # finish trailing by one tile
                        pp_ = list(proj_part_units(s))
                        pf = list(proj_fin_units(s))
                        units.extend(pp_)
                        for j in range(4):
                            units.append(av[j])
                            if j > 0:
                                units.append(pf[j - 1])
                        units.append(pf[3])
                    else:
                        units += av
                return units

            def emit_with_extras(primary, extra_specs):
                # extra_specs: (unit, frac) — run unit after frac*len(primary)
                n = len(primary)
                at = {}
                for u, f in extra_specs:
                    at.setdefault(min(n, max(0, int(f * n))), []).append(u)
                for i, u in enumerate(primary):
                    for eu in at.get(i, ()):
                        eu()
                    u()
                for eu in at.get(n, ()):
                    eu()

            # ================= emission =================
            _q_stage, _k_stage, _yT_stage, _r_stage = {}, {}, {}, {}
            _opart_stage = {}
            _ss_tiles, _inv_tiles = {}, {}
            for hg in range(2 * NG):
                ss = stst.tile([P, 2, HL + 2], fp32, tag="ss", name=f"ss{hg}")
                _ss_tiles[hg] = ss

            # Strip order: big-exp strips in the middle so ACT saturates,
            # tiny strip 0 last for a short drain tail.
            order = [0, 1, 2, 3]
            _first_strip = order[0]
            for u in group_units(0):
                u()
            for u in group_units(1):
                u()
            prepped = 2
            prev = None
            for idx, s in enumerate(order):
                extras = []
                nxt = order[idx + 1] if idx + 1 < len(order) else None
                if nxt is not None and nxt + 1 > prepped:
                    gu = []
                    for g in range(prepped, nxt + 1):
                        gu.extend(group_units(g))
                    prepped = nxt + 1
                    for i, u in enumerate(gu):
                        extras.append((u, 0.15 + 0.6 * i / len(gu)))
                if prev is not None:
                    pu = list(proj_units(prev))
                    for i, u in enumerate(pu):
                        extras.append((u, 0.1 + 0.85 * i / len(pu)))
                emit_with_extras(
                    strip_primary(s, inline_proj=(idx == len(order) - 1)),
                    extras,
                )
                prev = s

    nc.compile()
    return nc


def _get_program():
    if "nc" not in _CACHE:
        _CACHE["nc"] = _build_program()
    return _CACHE["nc"]


def make_in_maps(x, Wq, Wk, Wv, Wproj, q_gain):
    import ml_dtypes

    bf16 = ml_dtypes.bfloat16
    cos, sin = _rope_tables()
    cos = cos.astype(bf16)
    sin = sin.astype(bf16)
    in_maps = []
    for c in range(8):
        b, hh = c // 2, c % 2
        wq = Wq[hh * 512 : (hh + 1) * 512]          # [512, 1024]
        wk = Wk[hh * 128 : (hh + 1) * 128]          # [128, 1024]
        wv = Wv[hh * 128 : (hh + 1) * 128]          # [128, 1024]
        wkv = np.concatenate([wk, wv], axis=0)      # [256, 1024]
        wp = Wproj[:, hh * 512 : (hh + 1) * 512]    # [1024, 512]
        in_maps.append(
            {
                "xT": np.ascontiguousarray(x[b].T).astype(bf16),
                "wqT": np.ascontiguousarray(wq.T).astype(bf16),
                "wkvT": np.ascontiguousarray(wkv.T).astype(bf16),
                "wpT": np.ascontiguousarray(wp.T).astype(bf16),
                "cos": cos,
                "sin": sin,
                "gain": np.ascontiguousarray(
                    np.broadcast_to(q_gain[hh * 8 : (hh + 1) * 8], (P, HL))
                ).astype(bf16),
            }
        )
    return in_maps


def kernel(x, Wq, Wk, Wv, Wproj, q_gain):
    from concourse import bass_utils

    x = np.asarray(x, dtype=np.float32)
    Wq = np.asarray(Wq, dtype=np.float32)
    Wk = np.asarray(Wk, dtype=np.float32)
    Wv = np.asarray(Wv, dtype=np.float32)
    Wproj = np.asarray(Wproj, dtype=np.float32)
    q_gain = np.asarray(q_gain, dtype=np.float32)

    nc = _get_program()
    in_maps = make_in_maps(x, Wq, Wk, Wv, Wproj, q_gain)
    res = bass_utils.run_bass_kernel_spmd(
        nc, in_maps, core_ids=list(range(8)), trace=False
    )
    out = np.empty((B, T, D), dtype=np.float32)
    for b in range(B):
        out[b] = np.asarray(res.results[2 * b]["out"], np.float32) + np.asarray(
            res.results[2 * b + 1]["out"], np.float32
        )
    return out

